# revision 12
# baseline (speedup 1.0000x reference)
"""Multi-head attention on 8 TRN2 NeuronCores.

Sharding: 4-way data-parallel over batch x 2-way tensor-parallel over heads.
Core c handles batch (c // 2) and heads [8*(c%2), 8*(c%2)+8).

Per-core kernel (feature-major / transposed layouts throughout):
  xT   [1024, 2048]  (bf16, d-major)           -> SBUF
  Q^T, K^T = Wq/Wk slices @ xT + bias           [512, 2048] (e-major, bf16)
             (1/sqrt(dk) folded into Wq, bq on host)
  V    = x @ Wv^T slice + bias, token-major     [2048, 8 heads, 64+1]
         (65th column = ones -> softmax denominator comes free in ctx matmul)
  S^T[k,q] = K^T.T @ Q^T per head               (two heads packed in the
             128-row PE array via tile_position row groups, contraction=64)
  P = exp(S^T)  (scores are small: |S|<~3, so no max-subtraction needed)
  ctx^T[d,q] (+denom row) = V.T @ P             accumulated over 16 k-tiles
  ctx^T normalized by 1/denom (partition_broadcast + DVE multiply)
  outT_partial[e,t] = Wo^T slice.T @ ctx^T      [1024, 2048] f32 -> DRAM

Host: out[b] = (outT_core(2b) + outT_core(2b+1)).T + bo.
"""

import numpy as np
import ml_dtypes
from contextlib import ExitStack

import concourse.bass as bass
import concourse.bacc as bacc
import concourse.mybir as mybir
import concourse.tile as tile
from concourse.bass_utils import run_bass_kernel_spmd

D = 1024          # d_model
HEADS = 16
DK = 64           # head dim
B = 4             # batch
S = 2048          # sequence length
TP = 2            # tensor-parallel ways (over heads)
DP = 4            # data-parallel ways (over batch)
N_CORES = 8
EL = D // TP      # 512 local projection dims
HL = HEADS // TP  # 8 local heads
T = S             # tokens per core (one batch)
KT = D // 128     # 8 contraction tiles for projections
TT = T // 128     # 16 token tiles
NQ = T // 512     # 4 query tiles
NK = T // 128     # 16 key tiles

F32 = mybir.dt.float32
BF16 = mybir.dt.bfloat16
AF = mybir.ActivationFunctionType
ALU = mybir.AluOpType


def _bcast_ap(ap: bass.AP, parts: int) -> bass.AP:
    """Prepend a step-0 partition dim: broadcast a (partition-less or 1-row)
    AP across `parts` partitions for DMA."""
    return bass.AP(tensor=ap.tensor, offset=ap.offset, ap=[[0, parts]] + list(ap.ap))


def build_program() -> bass.Bass:
    nc = bacc.Bacc("TRN2", debug=False)

    xT = nc.dram_tensor("xT", [D, T], BF16, kind="ExternalInput").ap()
    wqT = nc.dram_tensor("wqT", [D, EL], BF16, kind="ExternalInput").ap()
    wkT = nc.dram_tensor("wkT", [D, EL], BF16, kind="ExternalInput").ap()
    wvT = nc.dram_tensor("wvT", [D, EL], BF16, kind="ExternalInput").ap()
    woT = nc.dram_tensor("woT", [EL, D], BF16, kind="ExternalInput").ap()
    bq = nc.dram_tensor("bq", [EL], BF16, kind="ExternalInput").ap()
    bk = nc.dram_tensor("bk", [EL], BF16, kind="ExternalInput").ap()
    bv = nc.dram_tensor("bv", [EL], BF16, kind="ExternalInput").ap()
    outT = nc.dram_tensor("outT", [D, T], F32, kind="ExternalOutput").ap()

    with ExitStack() as ctx:
        tc = ctx.enter_context(tile.TileContext(nc))
        const = ctx.enter_context(tc.tile_pool(name="const", bufs=1))
        xw = ctx.enter_context(tc.tile_pool(name="xw", bufs=1))
        qkv = ctx.enter_context(tc.tile_pool(name="qkv", bufs=1))
        expp = ctx.enter_context(tc.tile_pool(name="expp", bufs=3))
        stage = ctx.enter_context(tc.tile_pool(name="stage", bufs=3))
        psp = ctx.enter_context(tc.tile_pool(name="psp", bufs=2, space="PSUM"))
        ctxp = ctx.enter_context(tc.tile_pool(name="ctxp", bufs=4, space="PSUM"))
        drp = ctx.enter_context(tc.tile_pool(name="drp", bufs=3, space="DRAM"))

        # ---------------- loads ----------------
        xt_sb = xw.tile([128, KT, T], BF16)          # [p, kt, t]
        for kt in range(KT):
            nc.sync.dma_start(out=xt_sb[:, kt, :], in_=xT[kt * 128:(kt + 1) * 128, :])
        wq_sb = xw.tile([128, KT, EL], BF16)
        wk_sb = xw.tile([128, KT, EL], BF16)
        wv_sb = xw.tile([128, KT, EL], BF16)
        for w_sb, w_dram in ((wq_sb, wqT), (wk_sb, wkT), (wv_sb, wvT)):
            for kt in range(KT):
                nc.sync.dma_start(out=w_sb[:, kt, :], in_=w_dram[kt * 128:(kt + 1) * 128, :])
        wo_sb = xw.tile([128, EL // 128, D], BF16)
        for kt in range(EL // 128):
            nc.sync.dma_start(out=wo_sb[:, kt, :], in_=woT[kt * 128:(kt + 1) * 128, :])

        # biases live on one partition row; folded into the matmuls as a
        # K=1 rank-1 update (lhsT/rhs of ones), avoiding extra DVE ops
        bq_sb = const.tile([1, EL], BF16)
        nc.sync.dma_start(out=bq_sb, in_=_bcast_ap(bq, 1))
        bk_sb = const.tile([1, EL], BF16)
        nc.sync.dma_start(out=bk_sb, in_=_bcast_ap(bk, 1))
        bv_sb = const.tile([1, EL], BF16)
        nc.sync.dma_start(out=bv_sb, in_=_bcast_ap(bv, 1))
        ones_sb = const.tile([1, 512], BF16)
        nc.vector.memset(ones_sb, 1.0)

        # ---------------- Q/K projections (e-major) ----------------
        qt_sb = qkv.tile([128, EL // 128, T], BF16)   # [p(=e within hp), hp, t]
        kt_sb = qkv.tile([128, EL // 128, T], BF16)
        for w_sb, b_sb, dst in ((wq_sb, bq_sb, qt_sb), (wk_sb, bk_sb, kt_sb)):
            for hp in range(EL // 128):
                for nt2 in range(T // 1024):
                    ps = psp.tile([128, 1024], F32, tag="ps", name="ps")
                    for half in range(2):
                        t0 = (nt2 * 2 + half) * 512
                        # bias via rank-1 update: ps = b[e] * ones[t]
                        nc.tensor.matmul(
                            ps[:, half * 512:(half + 1) * 512],
                            lhsT=b_sb[0:1, hp * 128:(hp + 1) * 128],
                            rhs=ones_sb[0:1, :],
                            start=True, stop=False)
                        for kt in range(KT):
                            nc.tensor.matmul(
                                ps[:, half * 512:(half + 1) * 512],
                                lhsT=w_sb[:, kt, hp * 128:(hp + 1) * 128],
                                rhs=xt_sb[:, kt, t0:t0 + 512],
                                start=False, stop=(kt == KT - 1))
                    nc.vector.tensor_copy(
                        out=dst[:, hp, nt2 * 1024:(nt2 + 1) * 1024],
                        in_=ps)

        # ---------------- V projection (token-major) ----------------
        v_sb = qkv.tile([128, TT, HL, DK + 1], BF16)  # [p(=t within tt), tt, h, dk+ones]
        nc.vector.memset(v_sb[:, :, :, DK:DK + 1], 1.0)
        for tt in range(TT):
            psv = psp.tile([128, 512], F32, tag="ps", name="psv")
            # bias via rank-1 update: psv = ones[t] * bv[e]
            nc.tensor.matmul(
                psv, lhsT=ones_sb[0:1, 0:128], rhs=bv_sb[0:1, :],
                start=True, stop=False)
            for kt in range(KT):
                nc.tensor.matmul(
                    psv,
                    lhsT=xt_sb[:, kt, tt * 128:(tt + 1) * 128],
                    rhs=wv_sb[:, kt, :],
                    start=False, stop=(kt == KT - 1))
            nc.vector.tensor_copy(
                out=v_sb[:, tt, :, 0:DK],
                in_=psv.rearrange("p (h d) -> p h d", h=HL))

        # ---------------- attention + output projection ----------------
        ctxT_sb = qkv.tile([128, EL // 128, T], BF16)  # [p(=d within hp), hp, q]
        outT_r = outT.rearrange("(E p) t -> p E t", p=128)  # [128, 8, 2048]

        for qt in range(NQ):
            for hp in range(EL // 128):
                hA, hB = 2 * hp, 2 * hp + 1
                ctxA = ctxp.tile([128, 512], F32, tag="ctx", name="ctxA")
                ctxB = ctxp.tile([128, 512], F32, tag="ctx", name="ctxB")
                for kt in range(NK):
                    ps = psp.tile([128, 1024], F32, tag="ps", name="pss")
                    # S^T tiles for two heads packed into row-groups 0-63 / 64-127
                    nc.tensor.matmul(
                        ps[:, 0:512],
                        lhsT=kt_sb[0:64, hp, kt * 128:(kt + 1) * 128],
                        rhs=qt_sb[0:64, hp, qt * 512:(qt + 1) * 512],
                        start=True, stop=True)
                    nc.tensor.matmul(
                        ps[:, 512:1024],
                        lhsT=kt_sb[64:128, hp, kt * 128:(kt + 1) * 128],
                        rhs=qt_sb[64:128, hp, qt * 512:(qt + 1) * 512],
                        start=True, stop=True, tile_position=(64, 0))
                    es = expp.tile([128, 1024], BF16, tag="es", name="es")
                    nc.scalar.activation(out=es, in_=ps, func=AF.Exp)
                    nc.tensor.matmul(
                        ctxA[0:DK + 1, :], lhsT=v_sb[:, kt, hA, :], rhs=es[:, 0:512],
                        start=(kt == 0), stop=(kt == NK - 1))
                    nc.tensor.matmul(
                        ctxB[0:DK + 1, :], lhsT=v_sb[:, kt, hB, :], rhs=es[:, 512:1024],
                        start=(kt == 0), stop=(kt == NK - 1))
                for cps, h in ((ctxA, hA), (ctxB, hB)):
                    rec = stage.tile([1, 512], F32, tag="rec", name="rec")
                    nc.vector.reciprocal(rec, cps[DK:DK + 1, :])
                    rec_dr = drp.tile([1, 512], F32, tag="rec_dr", name="rec_dr")
                    nc.gpsimd.dma_start(out=rec_dr, in_=rec)
                    bc = stage.tile([DK, 512], F32, tag="bc", name="bc")
                    nc.gpsimd.dma_start(out=bc, in_=_bcast_ap(rec_dr[0, :], DK))
                    r0 = (h % 2) * DK
                    nc.vector.tensor_tensor(
                        out=ctxT_sb[r0:r0 + DK, hp, qt * 512:(qt + 1) * 512],
                        in0=cps[0:DK, :], in1=bc, op=ALU.mult)

            # output projection for this query tile
            for et2 in range(4):
                pso = psp.tile([128, 1024], F32, tag="ps", name="pso")
                for j in range(2):
                    et = et2 * 2 + j
                    for hp in range(EL // 128):
                        nc.tensor.matmul(
                            pso[:, j * 512:(j + 1) * 512],
                            lhsT=wo_sb[:, hp, et * 128:(et + 1) * 128],
                            rhs=ctxT_sb[:, hp, qt * 512:(qt + 1) * 512],
                            start=(hp == 0), stop=(hp == EL // 128 - 1))
                ot = stage.tile([128, 2, 512], F32, tag="ot", name="ot")
                nc.vector.tensor_copy(ot, pso.rearrange("p (a t) -> p a t", a=2))
                nc.sync.dma_start(
                    out=outT_r[:, et2 * 2:et2 * 2 + 2, qt * 512:(qt + 1) * 512],
                    in_=ot)

    nc.compile()
    return nc


_PROG = None


def _get_prog() -> bass.Bass:
    global _PROG
    if _PROG is None:
        _PROG = build_program()
    return _PROG


def make_in_maps(x, Wq, bq, Wk, bk, Wv, bv, Wo, bo):
    """Build the 8 per-core input dicts from the full (unsharded) inputs."""
    bf = ml_dtypes.bfloat16
    x = np.asarray(x, np.float32)
    scale = np.float32(1.0 / np.sqrt(DK))
    WqT = np.asarray(Wq, np.float32).T * scale   # [d, e], scores scale folded in
    WkT = np.asarray(Wk, np.float32).T
    WvT = np.asarray(Wv, np.float32).T
    WoT = np.asarray(Wo, np.float32).T           # [d_in, e_out]; rows = ctx dims
    bq = np.asarray(bq, np.float32) * scale
    bk = np.asarray(bk, np.float32)
    bv = np.asarray(bv, np.float32)

    xT_b = [np.ascontiguousarray(x[b_].T).astype(bf) for b_ in range(B)]
    in_maps = []
    for c in range(N_CORES):
        b_idx, h2 = divmod(c, TP)
        sl = slice(h2 * EL, (h2 + 1) * EL)
        in_maps.append({
            "xT": xT_b[b_idx],
            "wqT": np.ascontiguousarray(WqT[:, sl]).astype(bf),
            "wkT": np.ascontiguousarray(WkT[:, sl]).astype(bf),
            "wvT": np.ascontiguousarray(WvT[:, sl]).astype(bf),
            "woT": np.ascontiguousarray(WoT[sl, :]).astype(bf),
            "bq": np.ascontiguousarray(bq[sl]).astype(bf),
            "bk": np.ascontiguousarray(bk[sl]).astype(bf),
            "bv": np.ascontiguousarray(bv[sl]).astype(bf),
        })
    return in_maps


def assemble_output(results, bo):
    """Sum TP partials, transpose back to [B, S, D], add output bias."""
    bo32 = np.asarray(bo, np.float32)
    out = np.empty((B, S, D), np.float32)
    for b_idx in range(B):
        acc = results[TP * b_idx]["outT"] + results[TP * b_idx + 1]["outT"]
        out[b_idx] = acc.T + bo32
    return out


def kernel(x, Wq, bq, Wk, bk, Wv, bv, Wo, bo):
    nc = _get_prog()
    in_maps = make_in_maps(x, Wq, bq, Wk, bk, Wv, bv, Wo, bo)
    res = run_bass_kernel_spmd(nc, in_maps, core_ids=list(range(N_CORES)))
    return assemble_output(res.results, bo)
